# revision 1
# baseline (speedup 1.0000x reference)
"""GQA kernel for Trainium2: B=2,T=2048,E=2048,G=4,QPG=4,D=128, causal + sinusoidal PE.

Sharding: one core per (batch, kv-group) pair = 2*4 = 8 cores.
Each core computes q/k/v projections for its group, attention for its 4 query
heads, and a partial output projection (its group's 512 columns of wo);
partials are summed on the host.

Layout strategy (per core):
  - host passes x^T (f32r), so projections run as lhsT=weight-tile, rhs=xT-tile
    producing Q^T/K^T/V^T [d, t] directly.
  - scores are computed transposed: S^T[tk, tq] = K^T_tile.T @ Q^T, softmax'd
    without max subtraction (scores bounded, verified), exp'd into bf16 P^T
    tiles that feed the PV matmul directly as lhsT.
  - denominator comes free via a ones-column appended to V (N=129).
  - attention output [tq, d] is normalized via per-partition scale, then
    PE-transposed to [d, tq] to feed the wo matmul as lhsT.
"""
import sys

sys.path.insert(0, "/opt/trn_rl_repo")

import math
import numpy as np

B, T, E = 2, 2048, 2048
G, QPG, D = 4, 4, 128
NQ = QPG * D          # 512 q columns per group
NKV = 2 * D           # 256 kv columns per group
TT = T // 128         # 16 t-tiles
TB = T // 512         # 4 t-blocks
NE = E // 128         # 16 e-tiles
ISD = 1.0 / math.sqrt(D)

_compiled = None


def _build():
    from concourse import bacc, tile, mybir

    f32 = mybir.dt.float32
    f32r = mybir.dt.float32r
    bf16 = mybir.dt.bfloat16
    ADD = mybir.AluOpType.add
    MULT = mybir.AluOpType.mult
    EXP = mybir.ActivationFunctionType.Exp
    COPY = mybir.ActivationFunctionType.Copy
    IDENT = mybir.ActivationFunctionType.Identity

    nc = bacc.Bacc("TRN2", target_bir_lowering=False, debug=False, num_devices=8)

    xt_d = nc.dram_tensor("xt", [E, T], f32r, kind="ExternalInput")        # x^T
    wq_d = nc.dram_tensor("wq", [E, NQ], f32r, kind="ExternalInput")       # group slice
    wkv_d = nc.dram_tensor("wkv", [E, NKV], f32r, kind="ExternalInput")    # group slice
    wo_d = nc.dram_tensor("wo", [NQ, E], f32r, kind="ExternalInput")       # group slice
    pet_d = nc.dram_tensor("pet", [D, T], f32, kind="ExternalInput")       # pe^T
    bq_d = nc.dram_tensor("bq", [D, QPG], f32, kind="ExternalInput")       # col h
    bk_d = nc.dram_tensor("bk", [D, 1], f32, kind="ExternalInput")
    bv_d = nc.dram_tensor("bv", [D, 1], f32, kind="ExternalInput")
    msk_d = nc.dram_tensor("msk", [4, 128, 512], bf16, kind="ExternalInput")
    idf_d = nc.dram_tensor("idf", [128, 128], f32, kind="ExternalInput")
    idb_d = nc.dram_tensor("idb", [128, 128], bf16, kind="ExternalInput")
    ones_d = nc.dram_tensor("ones1", [128, 1], bf16, kind="ExternalInput")
    out_d = nc.dram_tensor("out", [T, E], f32, kind="ExternalOutput")

    with tile.TileContext(nc) as tc:
        with tc.tile_pool(name="persist", bufs=1) as pp:
            # ---- persistent tiles (DMAs for later-phase constants go on the
            # gpsimd queue so they don't delay phase-1's weight/xT stream) ----
            pet = pp.tile([D, T], f32)
            nc.gpsimd.dma_start(pet[:], pet_d[:])
            bq = pp.tile([D, QPG], f32)
            nc.gpsimd.dma_start(bq[:], bq_d[:])
            bk = pp.tile([D, 1], f32)
            nc.gpsimd.dma_start(bk[:], bk_d[:])
            bv = pp.tile([D, 1], f32)
            nc.gpsimd.dma_start(bv[:], bv_d[:])
            idf = pp.tile([128, 128], f32)
            nc.gpsimd.dma_start(idf[:], idf_d[:])
            idb = pp.tile([128, 128], bf16)
            nc.gpsimd.dma_start(idb[:], idb_d[:])
            ones1 = pp.tile([128, 1], bf16)
            nc.gpsimd.dma_start(ones1[:], ones_d[:])
            msk = [pp.tile([128, 512], bf16, name=f"msk{j}", tag=f"msk{j}") for j in range(4)]
            for j in range(4):
                nc.gpsimd.dma_start(msk[j][:], msk_d[j])

            qt = [pp.tile([128, T], f32r, name=f"qt{h}", tag=f"qt{h}") for h in range(QPG)]
            kt = pp.tile([128, T], f32r)
            vext = [pp.tile([128, 132], bf16, name=f"vx{i}", tag=f"vx{i}") for i in range(TT)]
            at = [pp.tile([128, T], f32r, name=f"at{h}", tag=f"at{h}") for h in range(QPG)]
            wo_sb = [pp.tile([128, E], f32r, name=f"wo{h}", tag=f"wo{h}") for h in range(QPG)]

            # ---- phase 1: projections ----
            with (
                tc.tile_pool(name="p1", bufs=1) as p1,
                tc.tile_pool(name="p1x", bufs=4) as p1x,
                tc.tile_pool(name="psA", bufs=1, space="PSUM") as psA,
                tc.tile_pool(name="ps1b", bufs=2, space="PSUM") as ps1b,
            ):
                wq_sb = [p1.tile([128, NQ], f32r, name=f"wq{e}", tag=f"wq{e}") for e in range(NE)]
                wkv_sb = [p1.tile([128, NKV], f32r, name=f"wkv{e}", tag=f"wkv{e}") for e in range(NE)]
                # interleave weight-tile loads with the first t-block's xT tiles so
                # the first matmul can start after ~1.5 MB of DMA, not 6 MB.
                xt0 = [p1x.tile([128, 512], f32r, name=f"xt0_{e}", tag="xt", bufs=18) for e in range(NE)]
                for e in range(NE):
                    nc.sync.dma_start(wq_sb[e][:], wq_d[e * 128:(e + 1) * 128, :])
                    nc.sync.dma_start(wkv_sb[e][:], wkv_d[e * 128:(e + 1) * 128, :])
                    nc.sync.dma_start(xt0[e][:], xt_d[e * 128:(e + 1) * 128, 0:512])

                for tb in range(TB):
                    ts = slice(tb * 512, (tb + 1) * 512)
                    qt_ps = psA.tile([128, 4 * 512], f32, name="qt_ps", tag="qt_ps")
                    kt_ps = psA.tile([128, 512], f32, name="kt_ps", tag="kt_ps")
                    vt_ps = psA.tile([128, 512], f32, name="vt_ps", tag="vt_ps")
                    for e in range(NE):
                        if tb == 0:
                            xt_t = xt0[e]
                        else:
                            xt_t = p1x.tile([128, 512], f32r, name="xt", tag="xt", bufs=18)
                            nc.sync.dma_start(xt_t[:], xt_d[e * 128:(e + 1) * 128, ts])
                        st = e == 0
                        sp = e == NE - 1
                        for h in range(QPG):
                            nc.tensor.matmul(
                                qt_ps[:, h * 512:(h + 1) * 512],
                                wq_sb[e][:, h * 128:(h + 1) * 128],
                                xt_t[:], start=st, stop=sp,
                            )
                        nc.tensor.matmul(kt_ps[:], wkv_sb[e][:, 0:128], xt_t[:], start=st, stop=sp)
                        nc.tensor.matmul(vt_ps[:], wkv_sb[e][:, 128:256], xt_t[:], start=st, stop=sp)
                    # drain: bias (in-place on psum) then += pe^T -> sbuf f32r
                    for h in range(QPG):
                        sl = qt_ps[:, h * 512:(h + 1) * 512]
                        nc.vector.tensor_tensor(sl, sl, bq[:, h:h + 1].to_broadcast([128, 512]), ADD)
                        nc.vector.tensor_tensor(qt[h][:, ts], sl, pet[:, ts], ADD)
                    nc.vector.tensor_tensor(kt_ps[:], kt_ps[:], bk[:].to_broadcast([128, 512]), ADD)
                    nc.vector.tensor_tensor(kt[:, ts], kt_ps[:], pet[:, ts], ADD)
                    # v: bias then cast to bf16, then transpose each 128-tile
                    vtb = p1.tile([128, 512], bf16, name="vtb", tag="vtb")
                    nc.scalar.activation(vtb[:], vt_ps[:], IDENT, bias=bv[:], scale=1.0)
                    for i in range(4):
                        ti = tb * 4 + i
                        vtp = ps1b.tile([128, 128], bf16, name="vtp", tag="vtp")
                        nc.tensor.transpose(vtp[:], vtb[:, i * 128:(i + 1) * 128], idb[:])
                        nc.vector.tensor_copy(vext[ti][:, 0:128], vtp[:])
                        nc.vector.tensor_copy(vext[ti][:, 128:129], ones1[:])

            for h in range(QPG):
                nc.gpsimd.dma_start(wo_sb[h][:], wo_d[h * 128:(h + 1) * 128, :])

            # ---- phase 2+3: attention fused with output projection ----
            # Emission is software-pipelined. During a head's S^T score stretch
            # the in-order PE is paced by ACT's exp (~700ns/tile vs 228ns
            # matmul); wo-projection matmuls (f32r, immune to the FWL
            # last-matmul-fp32 guard) fill those gaps at ~1 unit per S^T tile.
            # PV runs as pure bf16 streaks (FWL keeps LDWEIGHTS at ~54ns) with
            # the DVE epilogues batched after each streak.
            with (
                tc.tile_pool(name="p2", bufs=17) as p2,
                tc.tile_pool(name="p2s", bufs=8) as p2s,
                tc.tile_pool(name="p3", bufs=3) as p3,
                tc.tile_pool(name="ps2", bufs=2, space="PSUM") as ps2,
            ):
                from collections import deque
                filler = deque()

                def drain(n):
                    for _ in range(n):
                        if not filler:
                            return
                        filler.popleft()()

                def wo_units(qb):
                    units = []
                    for jj in range(4):
                        ti = qb * 4 + jj
                        state = {}

                        def alloc(state=state):
                            state["o_sb"] = p3.tile([128, E], f32, name="osb", tag="osb")
                        units.append(alloc)
                        for eo in range(4):
                            def mmA(state=state, ti=ti, eo=eo):
                                w_ps = ps2.tile([128, 512], f32, name="w_ps", tag="mix", bufs=2)
                                state["w"] = w_ps
                                for h in range(2):
                                    nc.tensor.matmul(
                                        w_ps[:], at[h][:, ti * 128:(ti + 1) * 128],
                                        wo_sb[h][:, eo * 512:(eo + 1) * 512],
                                        start=(h == 0), stop=False,
                                    )

                            def mmB(state=state, ti=ti, eo=eo):
                                w_ps = state["w"]
                                for h in range(2, 4):
                                    nc.tensor.matmul(
                                        w_ps[:], at[h][:, ti * 128:(ti + 1) * 128],
                                        wo_sb[h][:, eo * 512:(eo + 1) * 512],
                                        start=False, stop=(h == 3),
                                    )
                                nc.vector.tensor_copy(state["o_sb"][:, eo * 512:(eo + 1) * 512], w_ps[:])
                            units.append(mmA)
                            units.append(mmB)

                        def store(state=state, ti=ti):
                            nc.sync.dma_start(out_d[ti * 128:(ti + 1) * 128, :], state["o_sb"][:])
                        units.append(store)
                    return units

                def make_pv_streak(h, qb, pt, and_then=None):
                    def emit():
                        # pure bf16 PV streak: all four tq sub-tiles back to back
                        o_list = []
                        for j in range(4):
                            tt = 4 * qb + j
                            o_ps = ps2.tile([128, 129], f32, name="o_ps", tag="o_ps", bufs=4)
                            o_list.append(o_ps)
                            for tk in range(tt + 1):
                                nc.tensor.matmul(
                                    o_ps[:], pt[tk][:, j * 128:(j + 1) * 128],
                                    vext[tk][:, 0:129],
                                    start=(tk == 0), stop=(tk == tt),
                                )
                        for j in range(4):
                            tt = 4 * qb + j
                            o_ps = o_list[j]
                            r_sb = p2s.tile([128, 1], f32, name="r", tag="r")
                            nc.vector.reciprocal(r_sb[:], o_ps[:, 128:129])
                            a_sb = p2s.tile([128, 128], f32, name="a", tag="a")
                            nc.vector.tensor_tensor(
                                a_sb[:], o_ps[:, 0:128], r_sb[:].to_broadcast([128, 128]), MULT,
                            )
                            at_ps = ps2.tile([128, 512], f32, name="at_ps", tag="mix", bufs=2)
                            nc.tensor.transpose(at_ps[:, 0:128], a_sb[:], idf[:])
                            nc.vector.tensor_copy(at[h][:, tt * 128:(tt + 1) * 128], at_ps[:, 0:128])
                        if and_then is not None:
                            and_then()
                    return emit

                pending_pv = None
                for qb in range(TB):
                    qs = slice(qb * 512, (qb + 1) * 512)
                    nkt = 4 * qb + 4
                    for h in range(QPG):
                        pt = []
                        for tk in range(nkt):
                            s_ps = ps2.tile([128, 512], f32, name="s_ps", tag="s_ps")
                            nc.tensor.matmul(
                                s_ps[:], kt[:, tk * 128:(tk + 1) * 128], qt[h][:, qs],
                                start=True, stop=True,
                            )
                            p_t = p2.tile([128, 512], bf16, name="pt", tag="pt")
                            nc.scalar.activation(p_t[:], s_ps[:], EXP, scale=ISD)
                            j = tk - 4 * qb
                            if j >= 0:
                                nc.vector.tensor_tensor(p_t[:], p_t[:], msk[j][:], MULT)
                            pt.append(p_t)
                            if tk == 1 and pending_pv is not None:
                                pending_pv()
                                pending_pv = None
                            else:
                                drain(1)
                        cb = None
                        if h == QPG - 1:
                            def cb(qb=qb):
                                filler.extend(wo_units(qb))
                        pending_pv = make_pv_streak(h, qb, pt, and_then=cb)
                if pending_pv is not None:
                    pending_pv()
                drain(len(filler) + 1)

    nc.compile()
    return nc


def _get_compiled():
    global _compiled
    if _compiled is None:
        _compiled = _build()
    return _compiled


def _host_inputs(x, wq, bq, wkv, bkv, wo):
    import jax.numpy as jnp

    pos = np.arange(T, dtype=np.float32)[:, None]
    i = np.arange(0, D, 2, dtype=np.float32)
    inv = np.exp(-(np.log(10000.0) * i / D))
    ang = pos * inv
    pe = np.zeros((T, D), np.float32)
    pe[:, 0::2] = np.sin(ang)
    pe[:, 1::2] = np.cos(ang)
    pet = np.ascontiguousarray(pe.T)

    # causal masks for the 4 diagonal tiles of a 512-wide tq block:
    # mask_j[p, c] = 1 if c >= 128*j + p
    c = np.arange(512)[None, :]
    p = np.arange(128)[:, None]
    msk = np.stack([(c >= 128 * j + p) for j in range(4)]).astype(np.float32)
    msk = np.asarray(jnp.asarray(msk, dtype=jnp.bfloat16))

    idf = np.eye(128, dtype=np.float32)
    idb = np.asarray(jnp.asarray(idf, dtype=jnp.bfloat16))
    ones1 = np.asarray(jnp.ones((128, 1), dtype=jnp.bfloat16))

    xts = [np.ascontiguousarray(x[b].T) for b in range(B)]
    in_maps = []
    for core in range(8):
        b, g = divmod(core, G)
        in_maps.append({
            "xt": xts[b],
            "wq": np.ascontiguousarray(wq[:, g * NQ:(g + 1) * NQ]),
            "wkv": np.ascontiguousarray(wkv[:, g * NKV:(g + 1) * NKV]),
            "wo": np.ascontiguousarray(wo[g * NQ:(g + 1) * NQ, :]),
            "pet": pet,
            "bq": np.ascontiguousarray(bq[g * NQ:(g + 1) * NQ].reshape(QPG, D).T),
            "bk": np.ascontiguousarray(bkv[g * NKV:g * NKV + D].reshape(D, 1)),
            "bv": np.ascontiguousarray(bkv[g * NKV + D:(g + 1) * NKV].reshape(D, 1)),
            "msk": msk,
            "idf": idf,
            "idb": idb,
            "ones1": ones1,
        })
    return in_maps


def run(x, wq, bq, wkv, bkv, wo, trace=False):
    from concourse.bass_utils import run_bass_kernel_spmd

    nc = _get_compiled()
    in_maps = _host_inputs(
        np.asarray(x, np.float32), np.asarray(wq, np.float32),
        np.asarray(bq, np.float32), np.asarray(wkv, np.float32),
        np.asarray(bkv, np.float32), np.asarray(wo, np.float32),
    )
    res = run_bass_kernel_spmd(nc, in_maps, core_ids=list(range(8)), trace=trace)
    out = np.zeros((B, T, E), np.float32)
    for core in range(8):
        b = core // G
        out[b] += res.results[core]["out"]
    return out, res


def kernel(x, wq, bq, wkv, bkv, wo):
    out, _ = run(x, wq, bq, wkv, bkv, wo, trace=False)
    return out



# revision 5
# speedup vs baseline: 1.0649x; 1.0649x over previous
"""GQA kernel for Trainium2: B=2,T=2048,E=2048,G=4,QPG=4,D=128, causal + sinusoidal PE.

Sharding: one core per (batch, kv-group) pair = 2*4 = 8 cores.
Each core computes q/k/v projections for its group, attention for its 4 query
heads, and a partial output projection (its group's 512 columns of wo);
partials are summed on the host.

v2 layout strategy (per core), all matmul operands bf16:
  - projections: weight tile stationary, x^T streams; per (tb, chain) one
    512-col psum chain over 16 e-tiles. pe^T+bias is folded host-side into
    petq/petk so each drain is a single DVE add.
  - scores S^T[tk, tq]: kt tile stationary, q^T streams 512 cols; exp'd on
    ACT into bf16 P^T tiles (no max subtraction; scores bounded).
  - PV is flipped vs v1: V tile [tk, d] is stationary and P^T streams 512
    cols, producing O^T[d, tq] directly in psum (no output transposes).
  - softmax denominators: R = sum_tk P^T accumulated on DVE, then gpsimd
    partition_all_reduce replicates the column sums to all partitions;
    reciprocal multiplies O^T elementwise during the psum drain.
  - phases are software-pipelined: the scalar engine's exp (~570ns/tile) is
    ~2.6x slower than a score matmul, so projection half-chains of block
    tb+1 and output-projection units of block qb-1 are interleaved into the
    scores stretch one unit per accumulated lag quantum.
"""
import sys

sys.path.insert(0, "/opt/trn_rl_repo")

import math
import numpy as np

B, T, E = 2, 2048, 2048
G, QPG, D = 4, 4, 128
NQ = QPG * D          # 512 q columns per group
NKV = 2 * D           # 256 kv columns per group
TT = T // 128         # 16 t-tiles
TB = T // 512         # 4 t-blocks
NE = E // 128         # 16 e-tiles
ISD = 1.0 / math.sqrt(D)

_compiled = None


def _build():
    from collections import deque
    from concourse import bacc, tile, mybir, bass_isa

    f32 = mybir.dt.float32
    bf16 = mybir.dt.bfloat16
    ADD = mybir.AluOpType.add
    MULT = mybir.AluOpType.mult
    EXP = mybir.ActivationFunctionType.Exp
    RADD = bass_isa.ReduceOp.add

    nc = bacc.Bacc("TRN2", target_bir_lowering=False, debug=False, num_devices=8)

    xt_d = nc.dram_tensor("xt", [E, T], bf16, kind="ExternalInput")         # x^T
    wq_d = nc.dram_tensor("wq", [E, NQ], bf16, kind="ExternalInput")        # group slice
    wkv_d = nc.dram_tensor("wkv", [E, NKV], bf16, kind="ExternalInput")     # group slice
    wo_d = nc.dram_tensor("wo", [NQ, E], bf16, kind="ExternalInput")        # group slice
    petq_d = nc.dram_tensor("petq", [QPG, D, T], bf16, kind="ExternalInput")  # pe^T + bq_h
    petk_d = nc.dram_tensor("petk", [D, T], bf16, kind="ExternalInput")       # pe^T + bk
    bv_d = nc.dram_tensor("bv", [D, 1], f32, kind="ExternalInput")
    msk_d = nc.dram_tensor("msk", [4, 128, 512], bf16, kind="ExternalInput")
    idb_d = nc.dram_tensor("idb", [128, 128], bf16, kind="ExternalInput")
    out_d = nc.dram_tensor("out", [T, E], bf16, kind="ExternalOutput")

    with tile.TileContext(nc) as tc:
        with (
            tc.tile_pool(name="main", bufs=1) as pp,
            tc.tile_pool(name="ps", bufs=1, space="PSUM") as ps,
        ):
            # ---- persistent constants (gpsimd DMA queue; phase-1 stream is
            # on sync so these never delay the first matmuls) ----
            petq = [pp.tile([D, T], bf16, name=f"petq{h}", tag=f"petq{h}") for h in range(QPG)]
            for h in range(QPG):
                nc.gpsimd.dma_start(petq[h][:], petq_d[h])
            petk = pp.tile([D, T], bf16)
            nc.gpsimd.dma_start(petk[:], petk_d[:])
            bv = pp.tile([D, 1], f32)
            nc.gpsimd.dma_start(bv[:], bv_d[:])
            idb = pp.tile([128, 128], bf16)
            nc.gpsimd.dma_start(idb[:], idb_d[:])
            msk = [pp.tile([128, 512], bf16, name=f"msk{j}", tag=f"msk{j}") for j in range(4)]
            for j in range(4):
                nc.gpsimd.dma_start(msk[j][:], msk_d[j])
            wo_sb = [pp.tile([128, E], bf16, name=f"wo{h}", tag=f"wo{h}") for h in range(QPG)]
            for h in range(QPG):
                nc.gpsimd.dma_start(wo_sb[h][:], wo_d[h * 128:(h + 1) * 128, :])

            # ---- persistent activations ----
            qt = [pp.tile([128, T], bf16, name=f"qt{h}", tag=f"qt{h}") for h in range(QPG)]
            kt = pp.tile([128, T], bf16)
            at = [pp.tile([128, T], bf16, name=f"at{h}", tag=f"at{h}") for h in range(QPG)]
            vxall = pp.tile([128, T], bf16, name="vxall", tag="vxall")

            # ---- phase-1 weights + x^T stream (sync queue) ----
            wq_sb = [pp.tile([128, NQ], bf16, name=f"wqs{e}", tag=f"wqs{e}") for e in range(NE)]
            wkv_sb = [pp.tile([128, NKV], bf16, name=f"wkvs{e}", tag=f"wkvs{e}") for e in range(NE)]
            xt_t = [[None] * NE for _ in range(TB)]
            for e in range(NE):
                nc.sync.dma_start(wq_sb[e][:], wq_d[e * 128:(e + 1) * 128, :])
                xt_t[0][e] = pp.tile([128, 512], bf16, name="xt", tag="xt", bufs=24)
                nc.sync.dma_start(xt_t[0][e][:], xt_d[e * 128:(e + 1) * 128, 0:512])
            for e in range(NE):
                nc.sync.dma_start(wkv_sb[e][:], wkv_d[e * 128:(e + 1) * 128, :])

            def emit_xt_dma(tb):
                ts = slice(tb * 512, (tb + 1) * 512)
                for e in range(NE):
                    xt_t[tb][e] = pp.tile([128, 512], bf16, name="xt", tag="xt", bufs=24)
                    nc.sync.dma_start(xt_t[tb][e][:], xt_d[e * 128:(e + 1) * 128, ts])

            # ---- unit generators; units are (cost_ns, fn) ----
            def proj_units(tb, c):
                # c: 0-3 = q head c, 4 = k, 5 = v; two half-chain units
                ts = slice(tb * 512, (tb + 1) * 512)
                state = {}

                def wsel(e):
                    if c < 4:
                        return wq_sb[e][:, c * 128:(c + 1) * 128]
                    if c == 4:
                        return wkv_sb[e][:, 0:128]
                    return wkv_sb[e][:, 128:256]

                def half_a():
                    ps_t = ps.tile([128, 512], f32, name="big", tag="big", bufs=3)
                    state["ps"] = ps_t
                    for e in range(8):
                        nc.tensor.matmul(ps_t[:], wsel(e), xt_t[tb][e][:],
                                         start=(e == 0), stop=False)

                def half_b():
                    ps_t = state["ps"]
                    for e in range(8, NE):
                        nc.tensor.matmul(ps_t[:], wsel(e), xt_t[tb][e][:],
                                         start=False, stop=(e == NE - 1))
                    if c < 4:
                        nc.vector.tensor_tensor(qt[c][:, ts], ps_t[:], petq[c][:, ts], ADD)
                    elif c == 4:
                        nc.vector.tensor_tensor(kt[:, ts], ps_t[:], petk[:, ts], ADD)
                    else:
                        vtb = pp.tile([128, 512], bf16, name="vtb", tag="vtb", bufs=2)
                        nc.vector.tensor_tensor(vtb[:], ps_t[:],
                                                bv[:].to_broadcast([128, 512]), ADD)
                        vtp = ps.tile([128, 512], bf16, name="vtp", tag="vtp", bufs=1)
                        for i in range(4):
                            nc.tensor.transpose(vtp[:, i * 128:(i + 1) * 128],
                                                vtb[:, i * 128:(i + 1) * 128], idb[:])
                        nc.vector.tensor_copy(vxall[:, ts], vtp[:])

                return [(1800, half_a), (2100, half_b)]

            def oproj_units(ti):
                state = {}

                def alloc():
                    state["o"] = pp.tile([128, E], bf16, name="osb", tag="osb", bufs=2)

                units = [(0, alloc)]
                for eo in range(4):
                    def one(eo=eo):
                        w_ps = ps.tile([128, 512], f32, name="big", tag="big", bufs=3)
                        for h in range(QPG):
                            nc.tensor.matmul(
                                w_ps[:], at[h][:, ti * 128:(ti + 1) * 128],
                                wo_sb[h][:, eo * 512:(eo + 1) * 512],
                                start=(h == 0), stop=(h == QPG - 1),
                            )
                        nc.vector.tensor_copy(state["o"][:, eo * 512:(eo + 1) * 512], w_ps[:])
                    units.append((1000, one))

                def store():
                    nc.sync.dma_start(out_d[ti * 128:(ti + 1) * 128, :], state["o"][:])
                units.append((0, store))
                return units

            # two filler queues: proj has a deadline (before next block's
            # scores), oproj is slack-filled
            fill_proj = deque()
            fill_oproj = deque()
            lag = [0]

            def absorb(extra):
                lag[0] += extra
                while lag[0] > 0 and (fill_proj or fill_oproj):
                    q = fill_proj if fill_proj else fill_oproj
                    cost, fn = q.popleft()
                    fn()
                    lag[0] -= cost

            # ---- phase-2 per block qb, heads in pairs ----
            def phase2_pair(qb, pair):
                qs = slice(qb * 512, (qb + 1) * 512)
                nkt = 4 * qb + 4
                pts = {h: [] for h in pair}
                R = {h: pp.tile([128, 512], bf16, name=f"R{h}", tag=f"R{h}", bufs=2)
                     for h in pair}
                for tk in range(nkt):
                    for h in pair:
                        s_ps = ps.tile([128, 512], f32, name="s", tag="s", bufs=2)
                        nc.tensor.matmul(s_ps[:], kt[:, tk * 128:(tk + 1) * 128],
                                         qt[h][:, qs], start=True, stop=True)
                        p_t = pp.tile([128, 512], bf16, name="pt", tag="pt", bufs=34)
                        nc.scalar.activation(p_t[:], s_ps[:], EXP, scale=ISD)
                        j = tk - 4 * qb
                        if j >= 0:
                            nc.vector.tensor_tensor(p_t[:], p_t[:], msk[j][:], MULT)
                        if tk == 0:
                            nc.vector.tensor_copy(R[h][:], p_t[:])
                        else:
                            nc.vector.tensor_tensor(R[h][:], R[h][:], p_t[:], ADD)
                        pts[h].append(p_t)
                        absorb(350)
                rcp = {}
                for h in pair:
                    allR = pp.tile([128, 512], f32, name="allR", tag="allR", bufs=2)
                    nc.gpsimd.partition_all_reduce(allR[:], R[h][:], 128, RADD)
                    r = pp.tile([128, 512], f32, name=f"rcp{h}", tag=f"rcp{h}", bufs=2)
                    nc.vector.reciprocal_approx_fast(r[:], allR[:])
                    rcp[h] = r
                o_ps = {h: ps.tile([128, 512], f32, name="o", tag="o", bufs=2)
                        for h in pair}
                for tk in range(nkt):
                    for h in pair:
                        nc.tensor.matmul(o_ps[h][:], vxall[:, tk * 128:(tk + 1) * 128],
                                         pts[h][tk][:],
                                         start=(tk == 0), stop=(tk == nkt - 1))
                for h in pair:
                    nc.vector.tensor_tensor(at[h][:, qs], o_ps[h][:], rcp[h][:], MULT)

            # ---- drive ----
            for c in range(6):
                for _, fn in proj_units(0, c):
                    fn()

            for qb in range(TB):
                if qb + 1 < TB:
                    emit_xt_dma(qb + 1)
                    for c in range(6):
                        fill_proj.extend(proj_units(qb + 1, c))
                phase2_pair(qb, (0, 1))
                phase2_pair(qb, (2, 3))
                # proj chains for tb=qb+1 must land before scores(qb+1)
                while fill_proj:
                    fill_proj.popleft()[1]()
                for ti in range(4 * qb, 4 * qb + 4):
                    fill_oproj.extend(oproj_units(ti))
            while fill_oproj:
                fill_oproj.popleft()[1]()

    nc.compile()
    return nc


def _get_compiled():
    global _compiled
    if _compiled is None:
        _compiled = _build()
    return _compiled


def _host_inputs(x, wq, bq, wkv, bkv, wo):
    import jax.numpy as jnp

    def to_bf16(a):
        return np.asarray(jnp.asarray(a, dtype=jnp.bfloat16))

    pos = np.arange(T, dtype=np.float32)[:, None]
    i = np.arange(0, D, 2, dtype=np.float32)
    inv = np.exp(-(np.log(10000.0) * i / D))
    ang = pos * inv
    pe = np.zeros((T, D), np.float32)
    pe[:, 0::2] = np.sin(ang)
    pe[:, 1::2] = np.cos(ang)
    pet = np.ascontiguousarray(pe.T)                       # [D, T]

    # causal masks for the 4 diagonal tiles of a 512-wide tq block:
    # mask_j[p, c] = 1 if c >= 128*j + p
    c = np.arange(512)[None, :]
    p = np.arange(128)[:, None]
    msk = to_bf16(np.stack([(c >= 128 * j + p) for j in range(4)]).astype(np.float32))
    idb = to_bf16(np.eye(128, dtype=np.float32))

    xts = [to_bf16(x[b].T) for b in range(B)]
    in_maps = []
    for core in range(8):
        b, g = divmod(core, G)
        bq_g = bq[g * NQ:(g + 1) * NQ].reshape(QPG, D)     # [h, d]
        petq = np.stack([pet + bq_g[h][:, None] for h in range(QPG)])  # [h, D, T]
        petk = pet + bkv[g * NKV:g * NKV + D][:, None]
        in_maps.append({
            "xt": xts[b],
            "wq": to_bf16(wq[:, g * NQ:(g + 1) * NQ]),
            "wkv": to_bf16(wkv[:, g * NKV:(g + 1) * NKV]),
            "wo": to_bf16(wo[g * NQ:(g + 1) * NQ, :]),
            "petq": to_bf16(petq),
            "petk": to_bf16(petk),
            "bv": np.ascontiguousarray(
                bkv[g * NKV + D:(g + 1) * NKV].reshape(D, 1)).astype(np.float32),
            "msk": msk,
            "idb": idb,
        })
    return in_maps


def run(x, wq, bq, wkv, bkv, wo, trace=False):
    from concourse.bass_utils import run_bass_kernel_spmd

    nc = _get_compiled()
    in_maps = _host_inputs(
        np.asarray(x, np.float32), np.asarray(wq, np.float32),
        np.asarray(bq, np.float32), np.asarray(wkv, np.float32),
        np.asarray(bkv, np.float32), np.asarray(wo, np.float32),
    )
    res = run_bass_kernel_spmd(nc, in_maps, core_ids=list(range(8)), trace=trace)
    out = np.zeros((B, T, E), np.float32)
    for core in range(8):
        b = core // G
        out[b] += np.asarray(res.results[core]["out"], dtype=np.float32)
    return out, res


def kernel(x, wq, bq, wkv, bkv, wo):
    out, _ = run(x, wq, bq, wkv, bkv, wo, trace=False)
    return out
